# revision 101
# baseline (speedup 1.0000x reference)
"""Trainium2 Bass kernel for windowed attention with LoRA + decomposed rel-pos bias.

Full-input contract: kernel(**inputs) takes the unsharded numpy inputs and
returns the full (64, 14, 14, 768) float32 output.

Strategy (8 NeuronCores, data-parallel over the 64-window batch, 8 windows/core):
  Host prep (numpy):
    - Fold LoRA into qkv weights:  Wq += lb_q@la_q, Wv += lb_v@la_v  (exact math).
    - Fold attention scale (2^-3, exact) into Wq / b_q; rel-pos tables get 1/scale.
    - Pre-transpose all weights + x so every on-chip matmul operand has its
      contraction dim on SBUF partitions (no on-chip transposes at all).
    - Gather rel_pos tables with the (q-k) index map; cast everything to bf16.
  On chip (per core, all SBUF resident), software-pipelined head-pair major:
    attention for head pair hh runs while TensorE filler work for later
    pairs (q projected two pairs ahead, k chunks, rel-pos features, v
    windows just in time) is emitted into the gap between each window's
    QK+exp and AV stages, keeping every engine's in-order queue busy.
    PSUM pools are split (attention ring / projection ring / rel ring) so
    ring recycling of one stream never stalls the other.
    - augmented q/k tiles [128, 196] per (window, head): one matmul per key
      chunk yields q@k^T*scale + rel_h + rel_w in PSUM (K-augmentation); the
      one-hot/zero filler rows are built on-chip (Pool memsets + one DVE
      broadcast from small DMA'd masters).
    - single fused exp per (window, head) over both key chunks (the chunk-1
      QK matmul overruns its lhsT into the next pair slot so all 128 psum
      rows are defined; junk rows land in unused a_sb rows).
    - attn@v with the ones-column appended to v (M=65) so the softmax
      denominator falls out of the same matmul; one VectorE reciprocal per
      (window, pair) extracts it, a DRAM-bounce DMA broadcasts it across
      partitions, one Pool multiply per window-half normalizes.
"""

import numpy as np
import ml_dtypes

B_TOTAL = 64
NCORES = 8
BPC = B_TOTAL // NCORES  # windows per core
H = W = 14
N = H * W  # 196 tokens per window
DIM = 768
NH = 12
HD = 64
DC = DIM // 128  # 6 contraction chunks
NKT0, NKT1 = 128, N - 128  # key-token chunks (128 + 68)
SCALE = HD ** -0.5  # 0.125, exact power of two

# row maps inside the 128-partition augmented q/k tiles
# even head parity: q/k rows 0:64, relh/kh-onehot 64:78, zeros 78:96,
#                   relw/kw-onehot 96:110; contraction range [0:110)
# odd  head parity: relw/kw-onehot 0:14, zeros 14:32, relh/kh-onehot 32:46,
#                   zeros 46:64, q/k rows 64:128; contraction range [0:128)
K_EVEN = 110
K_ODD = 128

_NC_CACHE = {}


def build_module():
    from contextlib import ExitStack

    import concourse.tile as tile
    from concourse import bacc, mybir

    f32 = mybir.dt.float32
    bf16 = mybir.dt.bfloat16
    AF = mybir.ActivationFunctionType
    ALU = mybir.AluOpType

    nc = bacc.Bacc(
        "TRN2", target_bir_lowering=False, debug=False, num_devices=NCORES
    )

    T = BPC * N  # 1568 tokens per core

    xT = nc.dram_tensor("xT", [DIM, T], bf16, kind="ExternalInput").ap()
    wqk = nc.dram_tensor("wqk", [DIM, 2 * DIM], bf16, kind="ExternalInput").ap()
    wv = nc.dram_tensor("wv", [DIM, DIM], bf16, kind="ExternalInput").ap()
    pw = nc.dram_tensor("pw", [DIM, DIM], bf16, kind="ExternalInput").ap()
    bqk = nc.dram_tensor("bqk", [2 * DIM], f32, kind="ExternalInput").ap()
    bv = nc.dram_tensor("bv", [DIM], bf16, kind="ExternalInput").ap()
    bp = nc.dram_tensor("bp", [DIM], bf16, kind="ExternalInput").ap()
    relh = nc.dram_tensor("relh", [HD, N], bf16, kind="ExternalInput").ap()
    relw = nc.dram_tensor("relw", [HD, N], bf16, kind="ExternalInput").ap()
    oh_e = nc.dram_tensor("oh_e", [46, N], bf16, kind="ExternalInput").ap()
    oh_o = nc.dram_tensor("oh_o", [64, N], bf16, kind="ExternalInput").ap()
    out = nc.dram_tensor("out", [T, DIM], bf16, kind="ExternalOutput").ap()

    with tile.TileContext(nc) as tc, ExitStack() as ctx:
        singles = ctx.enter_context(tc.tile_pool(name="singles", bufs=1))
        ps_a = ctx.enter_context(tc.tile_pool(name="psa", bufs=3, space="PSUM"))
        ps_p = ctx.enter_context(tc.tile_pool(name="psp", bufs=3, space="PSUM"))
        psd = ctx.enter_context(tc.tile_pool(name="psd", bufs=1, space="PSUM"))
        attn_pool = ctx.enter_context(tc.tile_pool(name="attn", bufs=6))
        r_pool = ctx.enter_context(tc.tile_pool(name="rp", bufs=2))
        osb_pool = ctx.enter_context(tc.tile_pool(name="osb", bufs=3))
        rd_pool = ctx.enter_context(tc.tile_pool(name="rd", bufs=2, space="DRAM"))

        # ---- prologue DMA: x token-halves stream on the SP queue while the
        #      weights and small operands issue in parallel on the Act queue,
        #      ordered by first use so the first matmuls start early ----
        xT_sb = singles.tile([128, DC, T], bf16)
        xT_r = xT.rearrange("(c p) t -> p c t", p=128)
        TH = T // 2
        nc.sync.dma_start(out=xT_sb[:, 0:3, 0:TH], in_=xT_r[:, 0:3, 0:TH])
        nc.sync.dma_start(out=xT_sb[:, 3:6, 0:TH], in_=xT_r[:, 3:6, 0:TH])
        wqk_sb = singles.tile([128, DC, 2 * DIM], bf16)
        wqk_r = wqk.rearrange("(c p) o -> p c o", p=128)
        nc.scalar.dma_start(out=wqk_sb[:, 0, :], in_=wqk_r[:, 0, :])
        bqk_sb = singles.tile([128, 2 * DC], f32)
        nc.scalar.dma_start(out=bqk_sb[:], in_=bqk.rearrange("(c p) -> p c", p=128))
        nc.scalar.dma_start(out=wqk_sb[:, 1, :], in_=wqk_r[:, 1, :])
        for c in range(2, DC):
            nc.sync.dma_start(out=wqk_sb[:, c, :], in_=wqk_r[:, c, :])
        nc.sync.dma_start(out=xT_sb[:, 0:3, TH:T], in_=xT_r[:, 0:3, TH:T])
        nc.sync.dma_start(out=xT_sb[:, 3:6, TH:T], in_=xT_r[:, 3:6, TH:T])
        wv_sb = singles.tile([128, DC, DIM], bf16)
        nc.sync.dma_start(
            out=wv_sb[:], in_=wv.rearrange("(c p) o -> p c o", p=128)
        )
        bv_sb = singles.tile([128, DIM], bf16)
        nc.sync.dma_start(
            out=bv_sb[:], in_=bv.unsqueeze(0).broadcast_to([128, DIM])
        )
        pw_sb = singles.tile([128, DC, DIM], bf16)
        nc.sync.dma_start(
            out=pw_sb[:], in_=pw.rearrange("(c p) o -> p c o", p=128)
        )
        bp_sb = singles.tile([128, DIM], bf16)
        nc.sync.dma_start(
            out=bp_sb[:], in_=bp.unsqueeze(0).broadcast_to([128, DIM])
        )
        relh_sb = singles.tile([128, N], bf16)
        nc.scalar.dma_start(out=relh_sb[0:64, :], in_=relh)
        nc.scalar.dma_start(out=relh_sb[64:128, :], in_=relh)
        relw_sb = singles.tile([128, N], bf16)
        nc.scalar.dma_start(out=relw_sb[0:64, :], in_=relw)
        nc.scalar.dma_start(out=relw_sb[64:128, :], in_=relw)
        ohm_sb = singles.tile([64, 2, N], bf16)
        nc.scalar.dma_start(out=ohm_sb[0:46, 0, :], in_=oh_e)
        nc.scalar.dma_start(out=ohm_sb[0:64, 1, :], in_=oh_o)

        NPAIR = BPC * NH  # 96
        qaug = singles.tile([128, NPAIR, N], bf16)
        # kaug has one zeroed padding slot: the attention QK matmul for key
        # chunk 1 reads 128 lhsT columns (68 real + 60 overrun into the next
        # slot) so all 128 psum rows are written, letting one exp cover both
        # key chunks. The junk rows land in unused a_sb rows.
        kaug_p = singles.tile([128, NPAIR + 1, N], bf16)
        kaug = kaug_p[:, 0:NPAIR, :]
        kfl = kaug_p.rearrange("p s q -> p (s q)")
        # [t-chunk partitions, window, chunk, head, hd+ones]
        vall = singles.tile([128, BPC, 2, NH, HD + 1], bf16)
        o2_all = singles.tile([128, DC, T], bf16)

        nc.vector.memset(vall[:, :, :, :, HD : HD + 1], 1.0)
        nc.gpsimd.memset(kaug_p[:, NPAIR, :], 0.0)

        # views
        qv = qaug.rearrange(
            "p (b hh par) q -> p b hh par q", b=BPC, hh=NH // 2, par=2
        )
        qv6 = qaug.rearrange(
            "p (b hh par) (qh qw) -> p b hh par qh qw",
            b=BPC, hh=NH // 2, par=2, qh=H,
        )
        NPR = BPC * NH // 2  # 48 even/odd pair slots
        qpv = qaug.rearrange("p (pr par) q -> p pr par q", par=2)
        kpv = kaug.rearrange("p (pr par) q -> p pr par q", par=2)

        # zero filler rows via Pool memsets over 32-aligned partition ranges,
        # emitted BEFORE the writes (one-hot broadcast, rel feature copies)
        # that overwrite sub-ranges of them. kaug even rows 96:128 include
        # 110:128, which the overrunning chunk-1 QK lhsT reads.
        nc.gpsimd.memset(kpv[96:128, :, 0, :], 0.0)
        # one-hot filler rows: broadcast the DMA'd masters across the 48
        # pair slots on DVE (2x bf16 SBUF mode). The odd-slot broadcast has
        # no memset dependency, so it runs while Pool clears kaug even rows.
        nc.vector.tensor_copy(
            out=kpv[0:64, :, 1, :],
            in_=ohm_sb[0:64, 1:2, :].broadcast_to([64, NPR, N]),
        )
        nc.vector.tensor_copy(
            out=kpv[64:110, :, 0, :],
            in_=ohm_sb[0:46, 0:1, :].broadcast_to([46, NPR, N]),
        )
        nc.gpsimd.memset(qpv[64:96, :, 0, :], 0.0)
        nc.vector.memset(qpv[0:64, :, 1, :], 0.0)

        qp = qaug
        kp = kaug

        dest_v = [
            qaug.rearrange("p (b2 w2 h) q -> p b2 w2 h q", w2=2, h=NH),
            kaug.rearrange("p (b2 w2 h) q -> p b2 w2 h q", w2=2, h=NH),
        ]

        def make_qk_fill(oc, b2):
            """q/k projection tile split into two emission halves so gap
            filler interleaves at sub-500ns granularity."""
            state = {}

            def part1():
                p_qk = ps_p.tile([128, 512], f32, tag="psp", name="p_qk")
                state["p"] = p_qk
                for dc in range(3):
                    nc.tensor.matmul(
                        p_qk[:, 0 : 2 * N],
                        lhsT=wqk_sb[:, dc, oc * 128 : (oc + 1) * 128],
                        rhs=xT_sb[:, dc, 2 * b2 * N : (2 * b2 + 2) * N],
                        start=(dc == 0),
                        stop=False,
                    )

            def part2():
                p_qk = state["p"]
                for dc in range(3, DC):
                    nc.tensor.matmul(
                        p_qk[:, 0 : 2 * N],
                        lhsT=wqk_sb[:, dc, oc * 128 : (oc + 1) * 128],
                        rhs=xT_sb[:, dc, 2 * b2 * N : (2 * b2 + 2) * N],
                        start=False,
                        stop=(dc == DC - 1),
                    )
                hh = (oc % DC) * 2
                dv = dest_v[0] if oc < DC else dest_v[1]
                for par in range(2):
                    h = hh + par
                    rows = slice(0, 64) if par == 0 else slice(64, 128)
                    nc.scalar.activation(
                        out=dv[rows, b2, :, h, :],
                        in_=p_qk[rows, 0 : 2 * N].rearrange(
                            "p (w q) -> p w q", w=2
                        ),
                        func=AF.Identity,
                        bias=bqk_sb[rows, oc : oc + 1],
                        scale=1.0,
                    )

            return [part1, part2]

        def emit_proj_qk_tile(oc, b2):
            """q (oc<6) or k (oc>=6) projection, one 2-window psum tile."""
            p_qk = ps_p.tile([128, 512], f32, tag="psp")
            for dc in range(DC):
                nc.tensor.matmul(
                    p_qk[:, 0 : 2 * N],
                    lhsT=wqk_sb[:, dc, oc * 128 : (oc + 1) * 128],
                    rhs=xT_sb[:, dc, 2 * b2 * N : (2 * b2 + 2) * N],
                    start=(dc == 0),
                    stop=(dc == DC - 1),
                )
            hh = (oc % DC) * 2
            dv = dest_v[0] if oc < DC else dest_v[1]
            for par in range(2):
                h = hh + par
                rows = slice(0, 64) if par == 0 else slice(64, 128)
                nc.scalar.activation(
                    out=dv[rows, b2, :, h, :],
                    in_=p_qk[rows, 0 : 2 * N].rearrange("p (w q) -> p w q", w=2),
                    func=AF.Identity,
                    bias=bqk_sb[rows, oc : oc + 1],
                    scale=1.0,
                )

        def emit_rel_unit(hx, gq):
            """rel features for head hx, query-row block gq (up to 4 rows)."""
            par = hx % 2
            q_rows = slice(0, 64) if par == 0 else slice(64, 128)
            lh_base = 0 if par == 0 else 64
            relh_rows = slice(64, 78) if par == 0 else slice(32, 46)
            relw_rows = slice(96, 110) if par == 0 else slice(0, 14)
            relh_tp = (lh_base, 64 if par == 0 else 32)
            relw_tp = (lh_base, 96 if par == 0 else 0)
            hh, hp = hx // 2, hx % 2
            g0 = 4 * gq
            ng = min(4, H - g0)
            p_r = psd.tile([128, 8, 128], f32, tag="psd")
            for s in range(ng):
                g = g0 + s
                nc.tensor.matmul(
                    p_r[relh_rows, s, 0 : BPC * W],
                    lhsT=relh_sb[q_rows, g * W : (g + 1) * W],
                    rhs=qv[q_rows, :, hh, hp, g * W : (g + 1) * W],
                    start=True,
                    stop=True,
                    tile_position=relh_tp,
                )
                nc.tensor.matmul(
                    p_r[relw_rows, 4 + s, 0 : BPC * W],
                    lhsT=relw_sb[q_rows, g * W : (g + 1) * W],
                    rhs=qv[q_rows, :, hh, hp, g : g + 13 * W + 1 : W],
                    start=True,
                    stop=True,
                    tile_position=relw_tp,
                )
            if hx >= 8 and gq % 2 == 0:  # pairs 4-5: split between engines
                nc.scalar.activation(
                    out=qv6[relh_rows, :, hh, hp, g0 : g0 + ng, :],
                    in_=p_r[relh_rows, 0:ng, 0 : BPC * W].rearrange(
                        "p s (b w) -> p b s w", b=BPC
                    ),
                    func=AF.Copy,
                    scale=1.0,
                )
            else:
                nc.vector.tensor_copy(
                    out=qv6[relh_rows, :, hh, hp, g0 : g0 + ng, :],
                    in_=p_r[relh_rows, 0:ng, 0 : BPC * W].rearrange(
                        "p s (b w) -> p b s w", b=BPC
                    ),
                )
            nc.vector.tensor_copy(
                out=qv6[relw_rows, :, hh, hp, :, g0 : g0 + ng],
                in_=p_r[relw_rows, 4 : 4 + ng, 0 : BPC * W].rearrange(
                    "p s (b q) -> p b q s", b=BPC
                ),
            )

        def emit_qk_exp(b, hx, a_sb):
            par = hx % 2
            pair = b * NH + hx
            krange = slice(0, K_EVEN) if par == 0 else slice(0, K_ODD)
            p_a = ps_a.tile([128, 2, 256], f32, tag="psa")
            nc.tensor.matmul(
                p_a[:, 0, 0:N],
                lhsT=kp[krange, pair, 0:NKT0],
                rhs=qp[krange, pair, :],
                start=True,
                stop=True,
            )
            nc.tensor.matmul(
                p_a[:, 1, 0:N],
                lhsT=kfl[krange, pair * N + NKT0 : pair * N + NKT0 + 128],
                rhs=qp[krange, pair, :],
                start=True,
                stop=True,
            )
            # one fused exp over both key chunks; rows 68:128 of chunk 1 hold
            # exp of junk logits and land in unused a_sb rows.
            nc.scalar.activation(
                out=a_sb[:, :, :],
                in_=p_a[:, :, 0:N],
                func=AF.Exp,
                scale=1.0,
            )

        def emit_av(b, hx, a_sb, p_o):
            """attn @ [v | 1]: rows 0:64 = out, row 64 = softmax denominator."""
            par = hx % 2
            cols = slice(0, N) if par == 0 else slice(256, 256 + N)
            nc.tensor.matmul(
                p_o[0 : HD + 1, cols],
                lhsT=vall[0:NKT0, b, 0, hx, 0 : HD + 1],
                rhs=a_sb[:, 0, :],
                start=True,
                stop=False,
                skip_group_check=True,
            )
            nc.tensor.matmul(
                p_o[0 : HD + 1, cols],
                lhsT=vall[0:NKT1, b, 1, hx, 0 : HD + 1],
                rhs=a_sb[0:NKT1, 1, :],
                start=False,
                stop=True,
                skip_group_check=True,
            )

        def emit_proj_v_chunk(half, b, i):
            """v projection for one window token chunk (128 / 68 tokens)."""
            tc_rows = NKT0 if i == 0 else NKT1
            t0 = b * N + i * 128
            p_v = ps_p.tile([128, 512], f32, tag="psp")
            for dc in range(DC):
                nc.tensor.matmul(
                    p_v[0:tc_rows, 0:384],
                    lhsT=xT_sb[:, dc, t0 : t0 + tc_rows],
                    rhs=wv_sb[:, dc, half * 384 : (half + 1) * 384],
                    start=(dc == 0),
                    stop=(dc == DC - 1),
                )
            nc.vector.tensor_tensor(
                out=vall[0:tc_rows, b, i, 6 * half : 6 * half + 6, 0:HD],
                in0=p_v[0:tc_rows, 0:384].rearrange("p (h d) -> p h d", h=6),
                in1=bv_sb[0:tc_rows, half * 384 : (half + 1) * 384].rearrange(
                    "p (h d) -> p h d", h=6
                ),
                op=ALU.add,
            )

        def emit_proj_v_window(half, b):
            emit_proj_v_chunk(half, b, 0)
            emit_proj_v_chunk(half, b, 1)

        def emit_attn_qk(hh, b):
            h0, h1 = 2 * hh, 2 * hh + 1
            a_sb0 = attn_pool.tile([128, 2, N], bf16, tag="a0")
            a_sb1 = attn_pool.tile([128, 2, N], bf16, tag="a1")
            emit_qk_exp(b, h0, a_sb0)
            emit_qk_exp(b, h1, a_sb1)
            return a_sb0, a_sb1

        def emit_attn_av(hh, b, r_hh, asb):
            h0, h1 = 2 * hh, 2 * hh + 1
            a_sb0, a_sb1 = asb
            p_o = ps_a.tile([128, 512], f32, tag="psa")
            p_o2 = p_o.rearrange("p (s c) -> p s c", s=2)
            emit_av(b, h0, a_sb0, p_o)
            emit_av(b, h1, a_sb1, p_o)
            with nc.allow_low_precision(reason="bf16 softmax recip"):
                nc.vector.reciprocal(
                    out=r_hh[0:1, :, b, :],
                    in_=p_o2[HD : HD + 1, :, 0:N],
                )
            # psum->SBUF output copies: par0 on ScalarE; par1 alternates
            # ScalarE/VectorE by window parity to balance engine totals.
            if hh == 5:
                nc.vector.tensor_copy(
                    out=o2_all[0:64, hh, b * N : (b + 1) * N],
                    in_=p_o2[0:64, 0, 0:N],
                )
            else:
                nc.scalar.activation(
                    out=o2_all[0:64, hh, b * N : (b + 1) * N],
                    in_=p_o2[0:64, 0, 0:N],
                    func=AF.Copy,
                    scale=1.0,
                )
            if b % 2 == 0 or hh in (1, 2) or hh >= 4:
                nc.vector.tensor_copy(
                    out=o2_all[64:128, hh, b * N : (b + 1) * N],
                    in_=p_o2[0:64, 1, 0:N],
                )
            else:
                nc.scalar.activation(
                    out=o2_all[64:128, hh, b * N : (b + 1) * N],
                    in_=p_o2[0:64, 1, 0:N],
                    func=AF.Copy,
                    scale=1.0,
                )

        BH = BPC // 2

        def emit_bounce(hh, r_hh, half):
            """broadcast reciprocals for both parities of 4 windows."""
            b0 = half * BH
            dd = rd_pool.tile([2, BH, N], bf16, tag="rd")
            rb_hh = r_pool.tile([128, BH, N], bf16, tag="rb")
            bsl = slice(b0, b0 + BH)
            nc.sync.dma_start(out=dd[0:1, :, :], in_=r_hh[0:1, 0, bsl, :])
            nc.sync.dma_start(out=dd[1:2, :, :], in_=r_hh[0:1, 1, bsl, :])
            nc.sync.dma_start(
                out=rb_hh[0:64, :, :], in_=dd[0:1, :, :].broadcast_to([64, BH, N])
            )
            nc.sync.dma_start(
                out=rb_hh[64:128, :, :],
                in_=dd[1:2, :, :].broadcast_to([64, BH, N]),
            )
            return rb_hh

        def emit_normalize(hh, half, rb_hh):
            b0 = half * BH
            o2v = o2_all[:, hh, b0 * N : (b0 + BH) * N].rearrange(
                "p (b q) -> p b q", b=BH
            )
            eng = nc.vector if hh == 5 else nc.gpsimd
            eng.tensor_tensor(
                out=o2v, in0=o2v, in1=rb_hh[:, :, :], op=ALU.mult
            )

        p4_parked = {}

        def emit_p4_partial(j, half, p_p):
            """cc=0..4 of final-projection chunk (j, half); the psum chain
            stays open until the cc=5 matmul after the last normalize."""
            t0 = j * 128
            tc_rows = min(128, T - t0)
            for cc in range(DC - 1):
                nc.tensor.matmul(
                    p_p[0:tc_rows, 0:384],
                    lhsT=o2_all[:, cc, t0 : t0 + tc_rows],
                    rhs=pw_sb[:, cc, half * 384 : (half + 1) * 384],
                    start=(cc == 0),
                    stop=False,
                )
            p4_parked[(j, half)] = p_p

        # ---- software-pipelined main loop ----
        # Emission order IS the per-engine execution order, so each window's
        # attention block is split (qk+exp | filler | av+copies) and the
        # filler slot carries independent TensorE work for later pairs:
        # remaining k chunks, next pair's q, next pair's rel features, and
        # just-in-time v windows. attention for pair hh needs k chunks hh
        # and hh+1 (chunk-1 lhsT overrun) and v windows of its half.
        for b2 in range(BPC // 2):    # q pairs 0-1, k pairs 0-1; windows
            for oc in (0, DC, DC + 1, 1):  # in DMA-arrival order
                emit_proj_qk_tile(oc, b2)
        for gq in range((H + 3) // 4):
            emit_rel_unit(0, gq)
            emit_rel_unit(1, gq)
        emit_proj_v_window(0, 0)
        pending = []  # (hh, rb_hh) whose normalize is outstanding
        for hh in range(NH // 2):
            r_hh = r_pool.tile([1, 2, BPC, N], bf16, tag="rw")
            fill = []
            if hh < 4:  # q(hh+2): projected two pairs ahead so rel(hh+1)
                for b2 in range(BPC // 2):  # is eligible from the first gap
                    fill += make_qk_fill(hh + 2, b2)
            if hh < 4:  # k(hh+2), needed by attention hh+1's overrun
                for b2 in range(BPC // 2):
                    fill += make_qk_fill(DC + hh + 2, b2)
            rel_units = []
            if hh < 5:  # q(hh+1) is already done (projected at hh-1)
                rel_units = [
                    (lambda hx=hx, gq=gq: emit_rel_unit(hx, gq))
                    for gq in range((H + 3) // 4)
                    for hx in (2 * hh + 2, 2 * hh + 3)
                ]
            if hh == 3:
                emit_proj_v_window(1, 0)
            for b in range(BPC):
                asb = emit_attn_qk(hh, b)
                # ---- filler gap: independent TensorE work runs here while
                #      ScalarE computes this window's exp ----
                if hh == 5 and b == 0:
                    while pending:  # clear before parked phase-4 partials
                        emit_normalize(*pending.pop(0))
                if (hh == 0 or hh == 3) and b < BPC - 1:
                    emit_proj_v_chunk(hh // 3, b + 1, 0)
                for _ in range(4):
                    if rel_units:
                        rel_units.pop(0)()
                for _ in range(4):
                    if fill:
                        fill.pop(0)()
                if (hh == 0 or hh == 3) and b < BPC - 1:
                    emit_proj_v_chunk(hh // 3, b + 1, 1)
                if hh == 5 and b < 5:
                    jj, hf = divmod(b, 2)
                    if b < 3:
                        pt = ps_p.tile([128, 512], f32, tag="psp", name="p_pk")
                        emit_p4_partial(jj, hf, pt)
                    elif b == 3:
                        pd = psd.tile(
                            [128, 8, 128], f32, tag="psd", name="p_pkd"
                        )
                        pdf_ = pd.rearrange("p s c -> p (s c)")
                        emit_p4_partial(1, 1, pdf_[:, 0:512])
                        p4_psd2 = pdf_
                    else:
                        emit_p4_partial(2, 0, p4_psd2[:, 512:1024])
                if b in (1, 5) and pending:
                    emit_normalize(*pending.pop(0))
                if hh == 5 and b == 6 and pending:
                    emit_normalize(*pending.pop(0))
                emit_attn_av(hh, b, r_hh, asb)
                if b == BH - 1:
                    pending.append((hh, 0, emit_bounce(hh, r_hh, 0)))
            for f in fill + rel_units:
                f()
            pending.append((hh, 1, emit_bounce(hh, r_hh, 1)))
        for item in pending:
            emit_normalize(*item)

        # ---- final projection over global 128-token chunks ----
        # The first chunks were partially accumulated (cc=0..4) as TensorE
        # filler during the last attention pair; close them with the cc=5
        # matmul here, after o2[:,5,:] is normalized.
        NT_CH = (T + 127) // 128  # 13
        for j in range(NT_CH):
            t0 = j * 128
            tc_rows = min(128, T - t0)
            o_sb = osb_pool.tile([128, DIM], bf16, tag="osb")
            for half in range(2):
                parked = p4_parked.pop((j, half), None)
                if parked is None:
                    p_p = ps_p.tile([128, 512], f32, tag="psp")
                    cc0 = 0
                else:
                    p_p = parked
                    cc0 = DC - 1
                for cc in range(cc0, DC):
                    nc.tensor.matmul(
                        p_p[0:tc_rows, 0:384],
                        lhsT=o2_all[:, cc, t0 : t0 + tc_rows],
                        rhs=pw_sb[:, cc, half * 384 : (half + 1) * 384],
                        start=(cc == 0),
                        stop=(cc == DC - 1),
                    )
                nc.vector.tensor_tensor(
                    out=o_sb[0:tc_rows, half * 384 : (half + 1) * 384],
                    in0=p_p[0:tc_rows, 0:384],
                    in1=bp_sb[0:tc_rows, half * 384 : (half + 1) * 384],
                    op=ALU.add,
                )
            eng = nc.sync if j % 2 == 0 else nc.scalar
            eng.dma_start(
                out=out[t0 : t0 + tc_rows, :],
                in_=o_sb[0:tc_rows, :],
            )

    nc.finalize()
    return nc


def _host_prep(inputs):
    bf16 = ml_dtypes.bfloat16
    x = np.asarray(inputs["x"], np.float32)
    qkv_w = np.asarray(inputs["qkv_w"], np.float32)
    qkv_b = np.asarray(inputs["qkv_b"], np.float32)
    proj_w = np.asarray(inputs["proj_w"], np.float32)
    proj_b = np.asarray(inputs["proj_b"], np.float32)
    la_q = np.asarray(inputs["la_q"], np.float32)
    lb_q = np.asarray(inputs["lb_q"], np.float32)
    la_v = np.asarray(inputs["la_v"], np.float32)
    lb_v = np.asarray(inputs["lb_v"], np.float32)
    rel_pos_h = np.asarray(inputs["rel_pos_h"], np.float32)
    rel_pos_w = np.asarray(inputs["rel_pos_w"], np.float32)

    Wq = qkv_w[:DIM] + lb_q @ la_q
    Wk = qkv_w[DIM : 2 * DIM]
    Wv = qkv_w[2 * DIM :] + lb_v @ la_v

    wqk_host = np.ascontiguousarray(
        np.concatenate([SCALE * Wq, Wk], 0).T.astype(bf16)
    )
    wv_host = np.ascontiguousarray(Wv.T.astype(bf16))
    pw_host = np.ascontiguousarray(proj_w.T.astype(bf16))
    bqk_host = np.concatenate([SCALE * qkv_b[:DIM], qkv_b[DIM : 2 * DIM]]).astype(
        np.float32
    )
    bv_host = np.ascontiguousarray(qkv_b[2 * DIM :].astype(bf16))
    bp_host = np.ascontiguousarray(proj_b.astype(bf16))

    idx = np.arange(H)[:, None] - np.arange(H)[None, :] + (H - 1)
    Rh = rel_pos_h[idx]  # [qh, kh_j, hd]
    Rw = rel_pos_w[idx]  # [qw, kw_j, hd]
    relh_host = np.ascontiguousarray(
        (Rh / SCALE).transpose(2, 0, 1).reshape(HD, N).astype(bf16)
    )
    relw_host = np.ascontiguousarray(
        (Rw / SCALE).transpose(2, 0, 1).reshape(HD, N).astype(bf16)
    )

    kt = np.arange(N)
    oh_kh = (kt[None, :] // W == np.arange(H)[:, None]).astype(bf16)  # [14, 196]
    oh_kw = (kt[None, :] % W == np.arange(W)[:, None]).astype(bf16)
    z18 = np.zeros((18, N), bf16)
    oh_e_host = np.ascontiguousarray(np.concatenate([oh_kh, z18, oh_kw], 0))
    oh_o_host = np.ascontiguousarray(
        np.concatenate([oh_kw, z18, oh_kh, z18], 0)
    )

    shared = {
        "wqk": wqk_host,
        "wv": wv_host,
        "pw": pw_host,
        "bqk": bqk_host,
        "bv": bv_host,
        "bp": bp_host,
        "relh": relh_host,
        "relw": relw_host,
        "oh_e": oh_e_host,
        "oh_o": oh_o_host,
    }

    x_flat = x.reshape(B_TOTAL, N, DIM)
    in_maps = []
    for c in range(NCORES):
        xc = x_flat[c * BPC : (c + 1) * BPC].reshape(BPC * N, DIM)
        xT_c = np.ascontiguousarray(xc.T.astype(bf16))
        m = dict(shared)
        m["xT"] = xT_c
        in_maps.append(m)
    return in_maps


def kernel(**inputs):
    from concourse import bass_utils

    if "nc" not in _NC_CACHE:
        _NC_CACHE["nc"] = build_module()
    nc = _NC_CACHE["nc"]
    in_maps = _host_prep(inputs)
    res = bass_utils.run_bass_kernel_spmd(
        nc, in_maps, core_ids=list(range(NCORES))
    )
    outs = [
        r["out"].astype(np.float32).reshape(BPC, H, W, DIM) for r in res.results
    ]
    return np.concatenate(outs, 0)
